# revision 14
# baseline (speedup 1.0000x reference)
"""Trainium2 Bass kernel for DynamicSparseAttention (B=4, C=256, H=W=64).

Sharding: 2 cores per batch element (8 cores total); each core owns 2048 of
the 4096 queries and duplicates the (small) K/V work. Layouts are channel-
major so attention needs no transposes:
  - Q,K computed channel-major [C, N]; V token-major [N, C]
  - S^T tiles [keys=128, q=512] = K_tile.T @ Q  (keys on partitions)
  - softmax denominators: pairwise DVE tree over exp tiles + one
    ones-matmul for the cross-partition reduction (no max subtraction:
    |logits| < ~8, folded scale keeps exp in range)
  - O^T [C, q] = V_tile.T @ exp(S^T); normalization by gate/sum folded
    into the PSUM->SBUF copy of O; proj channel-major; residual add.

All matmuls bf16 (fp32 PSUM). fp8 DoubleRow was tried and measured ~2x
SLOWER per matmul on hardware (LDWEIGHTS for the 256-column interleaved
stationary is not hidden and FWL turns off), so everything stays bf16.

The core of this version is a 4-stage software pipeline over 512-query
chunks, so no engine ever waits on a cross-engine round trip:
  stage A (qc):   S matmuls + exp on ACT (paced by ACT, PE runs ahead)
  stage B (qc-1): AV matmuls (exp results a full chunk old - no stall)
  stage C (qc-2): denominator reciprocal chain + normalization of O
  stage D (qc-3): projection + residual + DMA out
Chunk 0's stage-B slots run the V matmuls instead. The gate stats (big
DVE reduces) are emitted after all Q/K PSUM casts so the DVE FIFO never
blocks the PE's head pipeline. x streams in as bf16 (residual error
~0.2% of |x|); gate sigmoids are computed via the Exp table so ACT never
reloads activation tables.
"""

import numpy as np
import ml_dtypes

import concourse.bass as bass
import concourse.bacc as bacc
import concourse.mybir as mybir
import concourse.tile as tile
from concourse.bass import ts
from concourse.bass_utils import run_bass_kernel_spmd

F32 = mybir.dt.float32
BF16 = mybir.dt.bfloat16
AF = mybir.ActivationFunctionType
ALU = mybir.AluOpType

B, C, H, W = 4, 256, 64, 64
N = H * W              # 4096 tokens per batch element
P = 128                # partitions
CT = C // P            # channel tiles (2)
NCORES = 8
QN = N * B // NCORES   # queries per core (2048)
HID = 32
QCH = 512              # query chunk for attention
MT = N // P            # key tiles (32)
NQC = QN // QCH        # query chunks per core (4)
NPAIR = MT // 2        # exp/AV pair slots per chunk (16)
SCALE = 1.0 / np.sqrt(C)   # folded into the exp activation scale
EXP_BIAS = -3.0            # cancels in normalization; keeps exp small

bf16 = ml_dtypes.bfloat16


def _build(reps=1, variant="full"):
    # variant flags for timing experiments (graded path always uses "full")
    no_attn = "noattn" in variant
    pstg_bufs = 3 if "ps3" in variant else 2
    from contextlib import ExitStack

    nc = bacc.Bacc()

    xb = nc.declare_dram_parameter("xb", [C, N], BF16, isOutput=False)
    wqT = nc.declare_dram_parameter("wqT", [C, C], BF16, isOutput=False)
    wkT = nc.declare_dram_parameter("wkT", [C, C], BF16, isOutput=False)
    wvT = nc.declare_dram_parameter("wvT", [C, C], BF16, isOutput=False)
    wpT = nc.declare_dram_parameter("wpT", [C, C], BF16, isOutput=False)
    wce1T = nc.declare_dram_parameter("wce1T", [C, HID], F32, isOutput=False)
    wce2T = nc.declare_dram_parameter("wce2T", [HID, 1], F32, isOutput=False)
    out = nc.declare_dram_parameter("out", [C, QN], F32, isOutput=True)

    xb_r = xb.rearrange("(t p) n -> p t n", p=P)
    wq_r = wqT.rearrange("(t p) o -> p t o", p=P)
    wk_r = wkT.rearrange("(t p) o -> p t o", p=P)
    wv_r = wvT.rearrange("(t p) o -> p t o", p=P)
    wp_r = wpT.rearrange("(t p) o -> p t o", p=P)
    wce1_r = wce1T.rearrange("(t p) h -> p t h", p=P)
    out_r = out.rearrange("(t p) n -> t p n", p=P)

    with tile.TileContext(nc) as tc:
        with (
            tc.tile_pool(name="cst", bufs=1) as cst,
            tc.tile_pool(name="ework", bufs=1) as ework,
            tc.tile_pool(name="work", bufs=2) as work,
            tc.tile_pool(name="ps", bufs=1, space="PSUM") as psum,
        ):
            # ---- weight loads + constants (outside the timing loop) ----
            wq_sb = cst.tile([P, CT, C], BF16)
            nc.sync.dma_start(wq_sb[:], wq_r[:])
            wk_sb = cst.tile([P, CT, C], BF16)
            nc.sync.dma_start(wk_sb[:], wk_r[:])
            wv_sb = cst.tile([P, CT, C], BF16)
            nc.sync.dma_start(wv_sb[:], wv_r[:])
            wp_sb = cst.tile([P, CT, C], BF16)
            nc.sync.dma_start(wp_sb[:], wp_r[:])
            wce1_sb = cst.tile([P, CT, HID], F32)
            nc.sync.dma_start(wce1_sb[:], wce1_r[:])
            wce2_sb = cst.tile([HID, 1], F32)
            nc.sync.dma_start(wce2_sb[:], wce2T[:])

            ones_bf = cst.tile([P, 1], BF16)
            nc.vector.memset(ones_bf[:], 1.0)
            onesrow = cst.tile([1, P], BF16)
            nc.vector.memset(onesrow[:], 1.0)
            ebias = cst.tile([P, 1], F32)
            nc.vector.memset(ebias[:], EXP_BIAS)

            _loop = ExitStack()
            if reps > 1:
                _loop.enter_context(tc.For_i(0, reps))
            # ---- x stream-in (bf16), double-buffered across reps ----
            NXC = 8
            XCH = N // NXC
            xb_sb = cst.tile([P, CT, N], BF16, tag="xb", bufs=2)
            feat = cst.tile([P, CT], F32)
            for j in range(NXC):
                for t in range(CT):
                    xeng = nc.gpsimd if t == 1 else nc.sync
                    xeng.dma_start(
                        xb_sb[:, t, ts(j, XCH)], xb_r[:, t, ts(j, XCH)]
                    )

            cmplx = cst.tile([1, 1], F32)

            def emit_gate():
                # gate stats + MLP. One big DVE reduce per (stat, t);
                # sigmoid via the Exp table: sigmoid(z) = 1/(1+exp(-z)),
                # so ACT never switches activation tables.
                for t in range(CT):
                    xm = work.tile([P, 1], F32, tag="xm")
                    nc.vector.reduce_max(
                        xm[:], xb_sb[:, t, :], axis=mybir.AxisListType.X
                    )
                    xs = work.tile([P, 1], F32, tag="xs")
                    nc.vector.reduce_sum(
                        xs[:], xb_sb[:, t, :], axis=mybir.AxisListType.X
                    )
                    nc.vector.scalar_tensor_tensor(
                        feat[:, t:t + 1], xs[:], 1.0 / N, xm[:],
                        op0=ALU.mult, op1=ALU.add,
                    )
                ph = psum.tile([HID, 1], F32, tag="pr", bufs=1, name="ph")
                for t in range(CT):
                    nc.tensor.matmul(
                        ph[:], wce1_sb[:, t, :], feat[:, t:t + 1],
                        start=(t == 0), stop=(t == CT - 1),
                    )
                e1 = cst.tile([HID, 1], F32)
                nc.scalar.activation(e1[:], ph[:], AF.Exp, scale=-1.0)
                d1 = cst.tile([HID, 1], F32)
                nc.vector.tensor_scalar_add(d1[:], e1[:], 1.0)
                r1 = cst.tile([HID, 1], F32)
                nc.vector.reciprocal(r1[:], d1[:])
                hid_sb = cst.tile([HID, 1], F32)
                nc.vector.tensor_tensor(hid_sb[:], ph[:], r1[:], op=ALU.mult)
                pc = psum.tile([1, 1], F32, tag="pr", bufs=1, name="pc")
                nc.tensor.matmul(pc[:], wce2_sb[:], hid_sb[:])
                e2 = cst.tile([1, 1], F32)
                nc.scalar.activation(e2[:], pc[:], AF.Exp, scale=-1.0)
                d2 = cst.tile([1, 1], F32)
                nc.vector.tensor_scalar_add(d2[:], e2[:], 1.0)
                nc.vector.reciprocal(cmplx[:], d2[:])

            # ---- Q then K (bf16 matmuls -> bf16 SBUF casts on DVE) ----
            q_sb = cst.tile([P, CT, QN], BF16)
            k_sb = cst.tile([P, CT, N], BF16)
            v_sb = cst.tile([P, MT, C], BF16)

            def q_chunk(j):
                for t in range(CT):
                    pq = psum.tile([P, QCH], F32, tag="po", bufs=2,
                                   name="pq")
                    for kc in range(CT):
                        nc.tensor.matmul(
                            pq[:], wq_sb[:, kc, ts(t, P)],
                            xb_sb[:, kc, ts(j, QCH)],
                            start=(kc == 0), stop=(kc == CT - 1),
                        )
                    if t == 0:
                        nc.vector.tensor_copy(q_sb[:, t, ts(j, QCH)], pq[:])
                    else:
                        nc.scalar.activation(q_sb[:, t, ts(j, QCH)], pq[:],
                                             AF.Copy)

            q_chunk(0)
            for j in range(N // QCH):
                for t in range(CT):
                    pk = psum.tile([P, QCH], F32, tag="po", bufs=2,
                                   name="pk")
                    for kc in range(CT):
                        nc.tensor.matmul(
                            pk[:], wk_sb[:, kc, ts(t, P)],
                            xb_sb[:, kc, ts(j, QCH)],
                            start=(kc == 0), stop=(kc == CT - 1),
                        )
                    if t == 0:
                        nc.vector.tensor_copy(k_sb[:, t, ts(j, QCH)], pk[:])
                    else:
                        nc.scalar.activation(k_sb[:, t, ts(j, QCH)], pk[:],
                                             AF.Copy)
            for j in range(1, NQC):
                q_chunk(j)

            # ---- attention: 4-stage pipeline over query chunks ----
            o_sb = cst.tile([P, CT, QN], BF16)
            et_tiles = {}
            po_tiles = {}
            pr_tiles = {}
            recipB_tiles = {}
            tree_tiles = {}

            def emit_v_slot(mp):
                # two token-tiles' V matmuls in chunk 0's AV slot mp
                pv2 = psum.tile([P, 2, C], F32, tag="po", bufs=2, name="pv")
                for hh in range(2):
                    nt = 2 * mp + hh
                    for kc in range(CT):
                        nc.tensor.matmul(
                            pv2[:, hh, :], xb_sb[:, kc, ts(nt, P)],
                            wv_sb[:, kc, :],
                            start=(kc == 0), stop=(kc == CT - 1),
                        )
                nc.vector.tensor_copy(v_sb[:, 2 * mp:2 * mp + 2, :], pv2[:])

            for qc in range(NQC + 3 if not no_attn else 0):
                c_av = qc - 1   # AV stage
                c_nm = qc - 2   # normalize stage
                c_pj = qc - 3   # projection stage
                for mp in range(NPAIR):
                    # ---- stage A: S + exp for chunk qc ----
                    if qc < NQC:
                        pstg = psum.tile([P, 2, QCH], F32, tag="ps2",
                                         bufs=pstg_bufs)
                        for h in range(2):
                            mt = 2 * mp + h
                            for kc in range(CT):
                                nc.tensor.matmul(
                                    pstg[:, h, :], k_sb[:, kc, ts(mt, P)],
                                    q_sb[:, kc, ts(qc, QCH)],
                                    start=(kc == 0), stop=(kc == CT - 1),
                                )
                        if mp == 0:
                            et_tiles[qc] = ework.tile(
                                [P, NPAIR, 2, QCH], BF16,
                                tag="exp", bufs=2, name="et",
                            )
                            tree_tiles[qc] = work.tile(
                                [P, NPAIR, QCH], BF16, tag="tree",
                                bufs=2, name="tree16",
                            )
                        et = et_tiles[qc]
                        nc.scalar.activation(
                            et[:, mp, :, :], pstg[:], AF.Exp,
                            bias=ebias[:], scale=float(SCALE),
                        )
                        # pairwise level-0 add for the denominator tree
                        nc.vector.tensor_tensor(
                            tree_tiles[qc][:, mp, :],
                            et[:, mp, 0, :], et[:, mp, 1, :], op=ALU.add,
                        )
                    # ---- stage B: AV for chunk c_av ----
                    if qc == 0:
                        emit_v_slot(mp)
                    elif 0 <= c_av < NQC:
                        etp = et_tiles[c_av]
                        if mp == 0:
                            po_tiles[c_av] = [
                                psum.tile([P, QCH], F32, tag="po", bufs=2,
                                          name=f"po{ct}")
                                for ct in range(CT)
                            ]
                        for h in range(2):
                            mt = 2 * mp + h
                            for ct in range(CT):
                                nc.tensor.matmul(
                                    po_tiles[c_av][ct][:],
                                    v_sb[:, mt, ts(ct, P)],
                                    etp[:, mp, h, :],
                                    start=(mt == 0), stop=(mt == MT - 1),
                                    skip_group_check=True,
                                )
                    if qc == 1 and mp == 8:
                        # stats + gate in iteration 1's DVE slack (first
                        # use of cmplx is iteration 2)
                        emit_gate()
                    # ---- stage B-tail: upper tree for c_av at mp==2
                    # (DVE-only); its ones-matmul deferred to mp==12 so
                    # the PE never waits on the fresh DVE tree adds ----
                    if mp == 2 and 0 <= c_av < NQC:
                        tr = tree_tiles[c_av]
                        w_half = NPAIR // 2
                        while w_half >= 1:
                            nc.vector.tensor_tensor(
                                tr[:, :w_half, :], tr[:, :w_half, :],
                                tr[:, w_half:2 * w_half, :], op=ALU.add,
                            )
                            w_half //= 2
                    if mp == 12 and 0 <= c_av < NQC:
                        prt = psum.tile([1, QCH], F32, tag="pr", bufs=1,
                                        name="pr")
                        pr_tiles[c_av] = prt
                        nc.tensor.matmul(prt[:], ones_bf[:],
                                         tree_tiles[c_av][:, 0, :])
                    # ---- stage C: reciprocal chain for c_nm at mp==4
                    # (denominator matmul ran at mp==12 last iteration);
                    # broadcast + normalize at mp==6 ----
                    if mp == 4 and 0 <= c_nm < NQC:
                        prt = pr_tiles[c_nm]
                        rr = work.tile([1, QCH], F32, tag="rr")
                        nc.vector.reciprocal(rr[:], prt[:])
                        rr2 = work.tile([1, QCH], BF16, tag="rr2", bufs=2)
                        nc.vector.tensor_scalar_mul(rr2[:], rr[:],
                                                    cmplx[:1, :1])
                        rr2_t = rr2
                        pr_tiles[(c_nm, 'rr2')] = rr2
                    if mp == 6 and 0 <= c_nm < NQC:
                        pb = psum.tile([P, QCH], F32, tag="pp", bufs=1,
                                       name="pb")
                        nc.tensor.matmul(pb[:], onesrow[:],
                                         pr_tiles[(c_nm, 'rr2')][:])
                        recipB = work.tile([P, QCH], F32, tag="recipB",
                                           bufs=2)
                        nc.scalar.activation(recipB[:], pb[:], AF.Copy)
                        recipB_tiles[c_nm] = recipB
                        for ct in range(CT):
                            nc.vector.tensor_tensor(
                                o_sb[:, ct, ts(c_nm, QCH)],
                                po_tiles[c_nm][ct][:], recipB[:],
                                op=ALU.mult,
                            )
                    # ---- stage D: projection + residual + out for c_pj,
                    # ct split across mp==10 / mp==13 so the single pp
                    # bank never stalls behind the outt add ----
                    if mp in (10, 13) and 0 <= c_pj < NQC:
                        ct = 0 if mp == 10 else 1
                        pp = psum.tile([P, QCH], F32, tag="pp", bufs=1)
                        for kc in range(CT):
                            nc.tensor.matmul(
                                pp[:], wp_sb[:, kc, ts(ct, P)],
                                o_sb[:, kc, ts(c_pj, QCH)],
                                start=(kc == 0), stop=(kc == CT - 1),
                            )
                        outt = work.tile([P, QCH], F32, tag="outt",
                                         bufs=3)
                        nc.vector.tensor_tensor(
                            outt[:], pp[:],
                            xb_sb[:, ct, ts(c_pj, QCH)], op=ALU.add,
                        )
                        nc.sync.dma_start(
                            out_r[ct, :, ts(c_pj, QCH)], outt[:]
                        )
            _loop.close()

    nc.finalize()
    return nc


_NC_CACHE = {}


def _get_nc():
    if "nc" not in _NC_CACHE:
        _NC_CACHE["nc"] = _build()
    return _NC_CACHE["nc"]


def _in_maps(x, w_ce1, w_ce2, wq, wk, wv, wproj):
    x = np.asarray(x, dtype=np.float32)
    wqT = np.ascontiguousarray(np.asarray(wq, np.float32).T).astype(bf16)
    wkT = np.ascontiguousarray(np.asarray(wk, np.float32).T).astype(bf16)
    wvT = np.ascontiguousarray(np.asarray(wv, np.float32).T).astype(bf16)
    wpT = np.ascontiguousarray(np.asarray(wproj, np.float32).T).astype(bf16)
    wce1T = np.ascontiguousarray(np.asarray(w_ce1, np.float32).T)
    wce2T = np.ascontiguousarray(np.asarray(w_ce2, np.float32).T)
    maps = []
    for c in range(NCORES):
        b, h = divmod(c, NCORES // B)
        xf = x[b].reshape(C, N)
        # keys ordered [my half | other half]; attention is permutation-
        # invariant over keys so any consistent order works
        xc = np.concatenate(
            [xf[:, h * QN:(h + 1) * QN], xf[:, (1 - h) * QN:(2 - h) * QN]],
            axis=1,
        )
        maps.append({
            "xb": np.ascontiguousarray(xc).astype(bf16),
            "wqT": wqT, "wkT": wkT, "wvT": wvT, "wpT": wpT,
            "wce1T": wce1T, "wce2T": wce2T,
        })
    return maps


def kernel(x, w_ce1, w_ce2, wq, wk, wv, wproj):
    x = np.asarray(x, dtype=np.float32)
    assert x.shape == (B, C, H, W)
    in_maps = _in_maps(x, w_ce1, w_ce2, wq, wk, wv, wproj)
    res = run_bass_kernel_spmd(_get_nc(), in_maps, list(range(NCORES)))
    out = np.empty((B, C, N), dtype=np.float32)
    for c in range(NCORES):
        b, h = divmod(c, NCORES // B)
        out[b][:, h * QN:(h + 1) * QN] = res.results[c]["out"]
    return out.reshape(B, C, H, W)


# revision 15
# speedup vs baseline: 1.1997x; 1.1997x over previous
"""Trainium2 Bass kernel for DynamicSparseAttention (B=4, C=256, H=W=64).

Sharding: 2 cores per batch element (8 cores total); each core owns 2048 of
the 4096 queries and duplicates the (small) K/V work. Layouts are channel-
major so attention needs no transposes:
  - Q,K computed channel-major [C, N]; V token-major [N, C]
  - S^T tiles [keys=128, q=512] = K_tile.T @ Q  (keys on partitions)
  - softmax denominators: pairwise DVE tree over exp tiles + one
    ones-matmul for the cross-partition reduction (no max subtraction:
    |logits| < ~8, folded scale keeps exp in range)
  - O^T [C, q] = V_tile.T @ exp(S^T); normalization by gate/sum folded
    into the PSUM->SBUF copy of O; proj channel-major; residual add.

All matmuls bf16 (fp32 PSUM). fp8 DoubleRow was tried and measured ~2x
SLOWER per matmul on hardware (LDWEIGHTS for the 256-column interleaved
stationary is not hidden and FWL turns off), so everything stays bf16.

The core of this version is a 4-stage software pipeline over 512-query
chunks, so no engine ever waits on a cross-engine round trip:
  stage A (qc):   S matmuls + exp on ACT (paced by ACT, PE runs ahead)
  stage B (qc-1): AV matmuls (exp results a full chunk old - no stall)
  stage C (qc-2): denominator reciprocal chain + normalization of O
  stage D (qc-3): projection + residual + DMA out
Chunk 0's stage-B slots run the V matmuls instead. The gate stats (big
DVE reduces) are emitted after all Q/K PSUM casts so the DVE FIFO never
blocks the PE's head pipeline. x streams in as bf16 (residual error
~0.2% of |x|); gate sigmoids are computed via the Exp table so ACT never
reloads activation tables.
"""

import numpy as np
import ml_dtypes

import concourse.bass as bass
import concourse.bacc as bacc
import concourse.mybir as mybir
import concourse.tile as tile
from concourse.bass import ts
from concourse.bass_utils import run_bass_kernel_spmd

F32 = mybir.dt.float32
BF16 = mybir.dt.bfloat16
AF = mybir.ActivationFunctionType
ALU = mybir.AluOpType

B, C, H, W = 4, 256, 64, 64
N = H * W              # 4096 tokens per batch element
P = 128                # partitions
CT = C // P            # channel tiles (2)
NCORES = 8
QN = N * B // NCORES   # queries per core (2048)
HID = 32
QCH = 512              # query chunk for attention
MT = N // P            # key tiles (32)
NQC = QN // QCH        # query chunks per core (4)
NPAIR = MT // 2        # exp/AV pair slots per chunk (16)
SCALE = 1.0 / np.sqrt(C)   # folded into the exp activation scale
EXP_BIAS = -3.0            # cancels in normalization; keeps exp small

bf16 = ml_dtypes.bfloat16


def _build(reps=1, variant="full"):
    # variant flags for timing experiments (graded path always uses "full")
    no_attn = "noattn" in variant
    pstg_bufs = 3 if "ps3" in variant else 2
    from contextlib import ExitStack

    nc = bacc.Bacc()

    xb = nc.declare_dram_parameter("xb", [C, N], BF16, isOutput=False)
    wqT = nc.declare_dram_parameter("wqT", [C, C], BF16, isOutput=False)
    wkT = nc.declare_dram_parameter("wkT", [C, C], BF16, isOutput=False)
    wvT = nc.declare_dram_parameter("wvT", [C, C], BF16, isOutput=False)
    wpT = nc.declare_dram_parameter("wpT", [C, C], BF16, isOutput=False)
    wce1T = nc.declare_dram_parameter("wce1T", [C, HID], F32, isOutput=False)
    wce2T = nc.declare_dram_parameter("wce2T", [HID, 1], F32, isOutput=False)
    out = nc.declare_dram_parameter("out", [C, QN], F32, isOutput=True)

    xb_r = xb.rearrange("(t p) n -> p t n", p=P)
    wq_r = wqT.rearrange("(t p) o -> p t o", p=P)
    wk_r = wkT.rearrange("(t p) o -> p t o", p=P)
    wv_r = wvT.rearrange("(t p) o -> p t o", p=P)
    wp_r = wpT.rearrange("(t p) o -> p t o", p=P)
    wce1_r = wce1T.rearrange("(t p) h -> p t h", p=P)
    out_r = out.rearrange("(t p) n -> t p n", p=P)

    with tile.TileContext(nc) as tc:
        with (
            tc.tile_pool(name="cst", bufs=1) as cst,
            tc.tile_pool(name="ework", bufs=1) as ework,
            tc.tile_pool(name="work", bufs=2) as work,
            tc.tile_pool(name="ps", bufs=1, space="PSUM") as psum,
        ):
            _loop = ExitStack()
            if reps > 1:
                _loop.enter_context(tc.For_i(0, reps))
            # ---- weight loads ----
            wq_sb = cst.tile([P, CT, C], BF16)
            nc.sync.dma_start(wq_sb[:], wq_r[:])
            wk_sb = cst.tile([P, CT, C], BF16)
            nc.sync.dma_start(wk_sb[:], wk_r[:])
            wv_sb = cst.tile([P, CT, C], BF16)
            nc.sync.dma_start(wv_sb[:], wv_r[:])
            wp_sb = cst.tile([P, CT, C], BF16)
            nc.sync.dma_start(wp_sb[:], wp_r[:])
            wce1_sb = cst.tile([P, CT, HID], F32)
            nc.sync.dma_start(wce1_sb[:], wce1_r[:])
            wce2_sb = cst.tile([HID, 1], F32)
            nc.sync.dma_start(wce2_sb[:], wce2T[:])

            ones_bf = cst.tile([P, 1], BF16)
            nc.vector.memset(ones_bf[:], 1.0)
            onesrow = cst.tile([1, P], BF16)
            nc.vector.memset(onesrow[:], 1.0)
            ebias = cst.tile([P, 1], F32)
            nc.vector.memset(ebias[:], EXP_BIAS)

            # ---- x stream-in (bf16) ----
            NXC = 8
            XCH = N // NXC
            xb_sb = cst.tile([P, CT, N], BF16)
            feat = cst.tile([P, CT], F32)
            for j in range(NXC):
                for t in range(CT):
                    xeng = nc.gpsimd if t == 1 else nc.sync
                    xeng.dma_start(
                        xb_sb[:, t, ts(j, XCH)], xb_r[:, t, ts(j, XCH)]
                    )

            cmplx = cst.tile([1, 1], F32)

            def emit_gate():
                # gate stats + MLP. One big DVE reduce per (stat, t);
                # sigmoid via the Exp table: sigmoid(z) = 1/(1+exp(-z)),
                # so ACT never switches activation tables.
                for t in range(CT):
                    xm = work.tile([P, 1], F32, tag="xm")
                    nc.vector.reduce_max(
                        xm[:], xb_sb[:, t, :], axis=mybir.AxisListType.X
                    )
                    xs = work.tile([P, 1], F32, tag="xs")
                    nc.vector.reduce_sum(
                        xs[:], xb_sb[:, t, :], axis=mybir.AxisListType.X
                    )
                    nc.vector.scalar_tensor_tensor(
                        feat[:, t:t + 1], xs[:], 1.0 / N, xm[:],
                        op0=ALU.mult, op1=ALU.add,
                    )
                ph = psum.tile([HID, 1], F32, tag="pr", bufs=1, name="ph")
                for t in range(CT):
                    nc.tensor.matmul(
                        ph[:], wce1_sb[:, t, :], feat[:, t:t + 1],
                        start=(t == 0), stop=(t == CT - 1),
                    )
                e1 = cst.tile([HID, 1], F32)
                nc.scalar.activation(e1[:], ph[:], AF.Exp, scale=-1.0)
                d1 = cst.tile([HID, 1], F32)
                nc.vector.tensor_scalar_add(d1[:], e1[:], 1.0)
                r1 = cst.tile([HID, 1], F32)
                nc.vector.reciprocal(r1[:], d1[:])
                hid_sb = cst.tile([HID, 1], F32)
                nc.vector.tensor_tensor(hid_sb[:], ph[:], r1[:], op=ALU.mult)
                pc = psum.tile([1, 1], F32, tag="pr", bufs=1, name="pc")
                nc.tensor.matmul(pc[:], wce2_sb[:], hid_sb[:])
                e2 = cst.tile([1, 1], F32)
                nc.scalar.activation(e2[:], pc[:], AF.Exp, scale=-1.0)
                d2 = cst.tile([1, 1], F32)
                nc.vector.tensor_scalar_add(d2[:], e2[:], 1.0)
                nc.vector.reciprocal(cmplx[:], d2[:])

            # ---- Q then K (bf16 matmuls -> bf16 SBUF casts on DVE) ----
            q_sb = cst.tile([P, CT, QN], BF16)
            k_sb = cst.tile([P, CT, N], BF16)
            v_sb = cst.tile([P, MT, C], BF16)

            def q_chunk(j):
                for t in range(CT):
                    pq = psum.tile([P, QCH], F32, tag="po", bufs=2,
                                   name="pq")
                    for kc in range(CT):
                        nc.tensor.matmul(
                            pq[:], wq_sb[:, kc, ts(t, P)],
                            xb_sb[:, kc, ts(j, QCH)],
                            start=(kc == 0), stop=(kc == CT - 1),
                        )
                    if t == 0:
                        nc.vector.tensor_copy(q_sb[:, t, ts(j, QCH)], pq[:])
                    else:
                        nc.scalar.activation(q_sb[:, t, ts(j, QCH)], pq[:],
                                             AF.Copy)

            q_chunk(0)
            for j in range(N // QCH):
                for t in range(CT):
                    pk = psum.tile([P, QCH], F32, tag="po", bufs=2,
                                   name="pk")
                    for kc in range(CT):
                        nc.tensor.matmul(
                            pk[:], wk_sb[:, kc, ts(t, P)],
                            xb_sb[:, kc, ts(j, QCH)],
                            start=(kc == 0), stop=(kc == CT - 1),
                        )
                    if t == 0:
                        nc.vector.tensor_copy(k_sb[:, t, ts(j, QCH)], pk[:])
                    else:
                        nc.scalar.activation(k_sb[:, t, ts(j, QCH)], pk[:],
                                             AF.Copy)
            for j in range(1, NQC):
                q_chunk(j)

            # ---- attention: 4-stage pipeline over query chunks ----
            o_sb = cst.tile([P, CT, QN], BF16)
            et_tiles = {}
            po_tiles = {}
            pr_tiles = {}
            recipB_tiles = {}
            tree_tiles = {}

            def emit_v_slot(mp):
                # two token-tiles' V matmuls in chunk 0's AV slot mp
                pv2 = psum.tile([P, 2, C], F32, tag="po", bufs=2, name="pv")
                for hh in range(2):
                    nt = 2 * mp + hh
                    for kc in range(CT):
                        nc.tensor.matmul(
                            pv2[:, hh, :], xb_sb[:, kc, ts(nt, P)],
                            wv_sb[:, kc, :],
                            start=(kc == 0), stop=(kc == CT - 1),
                        )
                nc.vector.tensor_copy(v_sb[:, 2 * mp:2 * mp + 2, :], pv2[:])

            for qc in range(NQC + 3 if not no_attn else 0):
                c_av = qc - 1   # AV stage
                c_nm = qc - 2   # normalize stage
                c_pj = qc - 3   # projection stage
                for mp in range(NPAIR):
                    # ---- stage A: S + exp for chunk qc ----
                    if qc < NQC:
                        pstg = psum.tile([P, 2, QCH], F32, tag="ps2",
                                         bufs=pstg_bufs)
                        for h in range(2):
                            mt = 2 * mp + h
                            for kc in range(CT):
                                nc.tensor.matmul(
                                    pstg[:, h, :], k_sb[:, kc, ts(mt, P)],
                                    q_sb[:, kc, ts(qc, QCH)],
                                    start=(kc == 0), stop=(kc == CT - 1),
                                )
                        if mp == 0:
                            et_tiles[qc] = ework.tile(
                                [P, NPAIR, 2, QCH], BF16,
                                tag="exp", bufs=2, name="et",
                            )
                            tree_tiles[qc] = work.tile(
                                [P, NPAIR, QCH], BF16, tag="tree",
                                bufs=2, name="tree16",
                            )
                        et = et_tiles[qc]
                        nc.scalar.activation(
                            et[:, mp, :, :], pstg[:], AF.Exp,
                            bias=ebias[:], scale=float(SCALE),
                        )
                        # pairwise level-0 add for the denominator tree
                        nc.vector.tensor_tensor(
                            tree_tiles[qc][:, mp, :],
                            et[:, mp, 0, :], et[:, mp, 1, :], op=ALU.add,
                        )
                    # ---- stage B: AV for chunk c_av ----
                    if qc == 0:
                        emit_v_slot(mp)
                    elif 0 <= c_av < NQC:
                        etp = et_tiles[c_av]
                        if mp == 0:
                            po_tiles[c_av] = [
                                psum.tile([P, QCH], F32, tag="po", bufs=2,
                                          name=f"po{ct}")
                                for ct in range(CT)
                            ]
                        for h in range(2):
                            mt = 2 * mp + h
                            for ct in range(CT):
                                nc.tensor.matmul(
                                    po_tiles[c_av][ct][:],
                                    v_sb[:, mt, ts(ct, P)],
                                    etp[:, mp, h, :],
                                    start=(mt == 0), stop=(mt == MT - 1),
                                    skip_group_check=True,
                                )
                    if qc == 1 and mp == 8:
                        # stats + gate in iteration 1's DVE slack (first
                        # use of cmplx is iteration 2)
                        emit_gate()
                    # ---- stage B-tail: upper tree for c_av at mp==2
                    # (DVE-only); its ones-matmul deferred to mp==12 so
                    # the PE never waits on the fresh DVE tree adds ----
                    if mp == 2 and 0 <= c_av < NQC:
                        tr = tree_tiles[c_av]
                        w_half = NPAIR // 2
                        while w_half >= 1:
                            nc.vector.tensor_tensor(
                                tr[:, :w_half, :], tr[:, :w_half, :],
                                tr[:, w_half:2 * w_half, :], op=ALU.add,
                            )
                            w_half //= 2
                    if mp == 12 and 0 <= c_av < NQC:
                        prt = psum.tile([1, QCH], F32, tag="pr", bufs=1,
                                        name="pr")
                        pr_tiles[c_av] = prt
                        nc.tensor.matmul(prt[:], ones_bf[:],
                                         tree_tiles[c_av][:, 0, :])
                    # ---- stage C: reciprocal chain for c_nm at mp==4
                    # (denominator matmul ran at mp==12 last iteration);
                    # broadcast + normalize at mp==6 ----
                    if mp == 4 and 0 <= c_nm < NQC:
                        prt = pr_tiles[c_nm]
                        rr = work.tile([1, QCH], F32, tag="rr")
                        nc.vector.reciprocal(rr[:], prt[:])
                        rr2 = work.tile([1, QCH], BF16, tag="rr2", bufs=2)
                        nc.vector.tensor_scalar_mul(rr2[:], rr[:],
                                                    cmplx[:1, :1])
                        rr2_t = rr2
                        pr_tiles[(c_nm, 'rr2')] = rr2
                    if mp == 6 and 0 <= c_nm < NQC:
                        pb = psum.tile([P, QCH], F32, tag="pp", bufs=1,
                                       name="pb")
                        nc.tensor.matmul(pb[:], onesrow[:],
                                         pr_tiles[(c_nm, 'rr2')][:])
                        recipB = work.tile([P, QCH], F32, tag="recipB",
                                           bufs=2)
                        nc.scalar.activation(recipB[:], pb[:], AF.Copy)
                        recipB_tiles[c_nm] = recipB
                        for ct in range(CT):
                            nc.vector.tensor_tensor(
                                o_sb[:, ct, ts(c_nm, QCH)],
                                po_tiles[c_nm][ct][:], recipB[:],
                                op=ALU.mult,
                            )
                    # ---- stage D: projection + residual + out for c_pj,
                    # ct split across mp==10 / mp==13 so the single pp
                    # bank never stalls behind the outt add ----
                    if mp in (10, 13) and 0 <= c_pj < NQC:
                        ct = 0 if mp == 10 else 1
                        pp = psum.tile([P, QCH], F32, tag="pp", bufs=1)
                        for kc in range(CT):
                            nc.tensor.matmul(
                                pp[:], wp_sb[:, kc, ts(ct, P)],
                                o_sb[:, kc, ts(c_pj, QCH)],
                                start=(kc == 0), stop=(kc == CT - 1),
                            )
                        outt = work.tile([P, QCH], F32, tag="outt",
                                         bufs=3)
                        nc.vector.tensor_tensor(
                            outt[:], pp[:],
                            xb_sb[:, ct, ts(c_pj, QCH)], op=ALU.add,
                        )
                        nc.sync.dma_start(
                            out_r[ct, :, ts(c_pj, QCH)], outt[:]
                        )
            _loop.close()

    nc.finalize()
    return nc


_NC_CACHE = {}


def _get_nc():
    if "nc" not in _NC_CACHE:
        _NC_CACHE["nc"] = _build()
    return _NC_CACHE["nc"]


def _in_maps(x, w_ce1, w_ce2, wq, wk, wv, wproj):
    x = np.asarray(x, dtype=np.float32)
    wqT = np.ascontiguousarray(np.asarray(wq, np.float32).T).astype(bf16)
    wkT = np.ascontiguousarray(np.asarray(wk, np.float32).T).astype(bf16)
    wvT = np.ascontiguousarray(np.asarray(wv, np.float32).T).astype(bf16)
    wpT = np.ascontiguousarray(np.asarray(wproj, np.float32).T).astype(bf16)
    wce1T = np.ascontiguousarray(np.asarray(w_ce1, np.float32).T)
    wce2T = np.ascontiguousarray(np.asarray(w_ce2, np.float32).T)
    maps = []
    for c in range(NCORES):
        b, h = divmod(c, NCORES // B)
        xf = x[b].reshape(C, N)
        # keys ordered [my half | other half]; attention is permutation-
        # invariant over keys so any consistent order works
        xc = np.concatenate(
            [xf[:, h * QN:(h + 1) * QN], xf[:, (1 - h) * QN:(2 - h) * QN]],
            axis=1,
        )
        maps.append({
            "xb": np.ascontiguousarray(xc).astype(bf16),
            "wqT": wqT, "wkT": wkT, "wvT": wvT, "wpT": wpT,
            "wce1T": wce1T, "wce2T": wce2T,
        })
    return maps


def kernel(x, w_ce1, w_ce2, wq, wk, wv, wproj):
    x = np.asarray(x, dtype=np.float32)
    assert x.shape == (B, C, H, W)
    in_maps = _in_maps(x, w_ce1, w_ce2, wq, wk, wv, wproj)
    res = run_bass_kernel_spmd(_get_nc(), in_maps, list(range(NCORES)))
    out = np.empty((B, C, N), dtype=np.float32)
    for c in range(NCORES):
        b, h = divmod(c, NCORES // B)
        out[b][:, h * QN:(h + 1) * QN] = res.results[c]["out"]
    return out.reshape(B, C, H, W)
